# revision 18
# baseline (speedup 1.0000x reference)
"""MoE-ALU (add with carry + xor over one-hot byte encodings) on 8 NeuronCores.

Semantics (validated against the jax reference bit-exactly): inputs a, b are
exact one-hot byte encodings [B, 4, 256] (little-endian bytes of 32-bit ints);
with SCALE=100 every softmax in the reference collapses to an exact one-hot, so

    out[0] = one_hot bytes of (a_int + b_int) mod 2^32
    out[1] = one_hot bytes of (a_int ^ b_int)

Device kernel (pure data parallel, batch sharded over 8 cores), raw Bass
(this toolchain's walrus encodes at most ONE sync wait per instruction, so
Tile-generated schedules don't compile; manual sems with standalone waits do).

The kernel is HBM-bound. The one-hot inputs are exactly representable in
bf16, so the host-side shard prep casts and interleaves a|b into one bf16
tensor: input traffic halves to 16 MB/core; output must stay f32 (32 MB).
Floor: ~48 MB/core at ~345 GB/s effective -> ~140 us of DMA wire time.

v5 structure -- two-tile pairs, software-pipelined DVE, bf16 input path:

  load    one 1 MB DMA per tile pair (both tiles' a|b rows, bf16)
  decode  per tile: 4x scalar_tensor_tensor with accum_out (multiply one
          512-col bf16 segment by the [0..255 | 0,256,...,65280] bf16
          pattern, reduce in one op; both streams 16-bit so the DVE runs
          packed) -> a_lo a_hi b_lo b_hi (16-bit halves, f32-exact)
  ints    per PAIR of tiles (strided APs halve the op count): one f32->i32
          cast [128,8], add -> [s_lo s_hi]x2, xor -> [x_lo x_hi]x2, carry
          folded in place into the high halves only ((s_lo>=2^16)+s_hi; the
          raw s_lo's bit 16 never survives the later >>0/>>8 then &255),
          shift/mask -> 16 byte indices
  encode  per tile: two is_equal [128, 4, 256] of the int iota table against
          stride-0-broadcast indices, writing f32 one-hots directly
  store   one 1 MB DMA per tile (both output planes via a strided DRAM AP)

  The emission order interleaves pair p's eight decode STTs between the
  dependent ops of pair p-1's int chain, so every RAW semaphore wait is
  already satisfied when the sequencer reaches it and the DVE never idles
  on sem latency (~130 ns per unsatisfied wait otherwise).

  engines: SyncE issues input DMAs (pair-0 data before the tables; pair
  slots released as soon as the decode STTs that read them retire, via
  static s_dve thresholds), ScalarE issues output DMAs, VectorE computes.
  ACT and GpSimd do no streaming work on purpose: measured on this part, a
  concurrent ACT stream slows every DVE op ~20%.

  DVE ops do NOT self-interlock (measured: removing sync gives stale reads),
  so every same-engine RAW step still waits on the monotonically counted DVE
  semaphore; per-pair temporaries are parity-double-buffered.
"""
from contextlib import ExitStack

import ml_dtypes
import numpy as np

import concourse.bass as bass
from concourse import mybir
from concourse.bass_utils import run_bass_kernel_spmd

F32 = mybir.dt.float32
I32 = mybir.dt.int32
BF16 = mybir.dt.bfloat16
NP_BF16 = ml_dtypes.bfloat16

P = 128
N_CORES = 8
B = 32768
B_LOC = B // N_CORES          # 4096 rows per core
ROW = 4 * 256                 # 1024 elements per row per tensor
N_TILES = B_LOC // P          # 32
N_PAIRS = N_TILES // 2        # 16

NPBUF = 6                     # input buffer slots (one tile PAIR each)
OBUF = 10                     # output buffer slots

TABI_COLS = 16                # shift pattern x2 tiles
TABF_COLS = 512 + 2048        # decode pattern | encode iota x8 (bf16)


def _schedule():
    """Emission order for the vector engine (single source of truth for the
    s_dve counter, shared by the sync/vector closures)."""
    ev = []
    ev += [("stt", 0, m) for m in range(8)]
    for p in range(1, N_PAIRS):
        q = p - 1
        ev += [
            ("stt", p, 0), ("cast", q),
            ("stt", p, 1), ("add", q),
            ("stt", p, 2), ("xor", q),
            ("stt", p, 3), ("isge", q),
            ("stt", p, 4), ("shift", q),
            ("stt", p, 5), ("and", q),
            ("stt", p, 6), ("castb", q),
            ("stt", p, 7), ("iseq", 2 * q, 0),
            ("iseq", 2 * q, 1),
            ("iseq", 2 * q + 1, 0), ("iseq", 2 * q + 1, 1),
        ]
    q = N_PAIRS - 1
    ev += [("cast", q), ("add", q), ("xor", q), ("isge", q), ("shift", q),
           ("and", q), ("castb", q), ("iseq", 2 * q, 0), ("iseq", 2 * q, 1),
           ("iseq", 2 * q + 1, 0), ("iseq", 2 * q + 1, 1)]

    after = {}
    n = 0
    for e in ev:
        if e[0] != "iseq":          # iseq increments s_comp, not s_dve
            n += 1
            after[e] = n
    return ev, after


EVENTS, AFTER = _schedule()
# input pair slot of pair p is free once its last decode STT retires
RELEASE_PAIR = {p: AFTER[("stt", p, 7)] for p in range(N_PAIRS)}


def _build_nc() -> bass.Bass:
    nc = bass.Bass(trn_type="TRN2")
    ab_d = nc.dram_tensor("ab", [B_LOC, 2 * ROW], BF16, kind="ExternalInput")
    tabf_d = nc.dram_tensor("tabf", [P, TABF_COLS], BF16, kind="ExternalInput")
    tabi_d = nc.dram_tensor("tabi", [P, TABI_COLS], I32, kind="ExternalInput")
    out_d = nc.dram_tensor("out", [2, B_LOC, ROW], F32, kind="ExternalOutput")

    with ExitStack() as ctx:
        sb = lambda name, shape, dt: ctx.enter_context(
            nc.sbuf_tensor(name, shape, dt))
        tabf_t = sb("tabf_t", [P, TABF_COLS], BF16)
        tabi_t = sb("tabi_t", [P, TABI_COLS], I32)
        abp_t = [sb(f"abp_t{k}", [P, 2 * 2 * ROW], BF16) for k in range(NPBUF)]
        out_t = [sb(f"out_t{k}", [P, 2 * ROW], BF16) for k in range(OBUF)]
        dump = [[sb(f"dump{c}_{k}", [P, 512], BF16) for k in range(8)]
                for c in range(2)]
        # parity-double-buffered per-pair temporaries
        t8 = [sb(f"t8_{c}", [P, 8], F32) for c in range(2)]
        iv8 = [sb(f"iv8_{c}", [P, 8], I32) for c in range(2)]
        v8 = [sb(f"v8_{c}", [P, 8], I32) for c in range(2)]
        sh16 = [sb(f"sh16_{c}", [P, 16], I32) for c in range(2)]
        idx16 = [sb(f"idx16_{c}", [P, 16], I32) for c in range(2)]
        idx16bf = [sb(f"idx16bf_{c}", [P, 16], BF16) for c in range(2)]

        dec = tabf_t[:, 0:512]
        enc = tabf_t[:, 512:2560].rearrange("p (e k) -> p e k", k=256)
        shifts = tabi_t[:].rearrange("p (a two) -> p a two", two=2)

        s_tab = ctx.enter_context(nc.semaphore("s_tab"))
        s_tab2 = ctx.enter_context(nc.semaphore("s_tab2"))
        s_la = [ctx.enter_context(nc.semaphore(f"s_la{j}"))
                for j in range(NPBUF)]
        s_lb = [ctx.enter_context(nc.semaphore(f"s_lb{j}"))
                for j in range(NPBUF)]
        s_store = [ctx.enter_context(nc.semaphore(f"s_store{j}"))
                   for j in range(OBUF)]
        s_comp = ctx.enter_context(nc.semaphore("s_comp"))
        s_dve = ctx.enter_context(nc.semaphore("s_dve"))

        block = ctx.enter_context(nc.Block())

        @block.sync
        def _(sync: bass.BassEngine):
            for p in range(N_PAIRS):
                jp = p % NPBUF
                if p >= NPBUF:
                    sync.wait_ge(s_dve, RELEASE_PAIR[p - NPBUF])
                r0 = 2 * P * p
                sync.dma_start(
                    out=abp_t[jp][:, 0:2 * ROW],
                    in_=ab_d[r0:r0 + P, :],
                ).then_inc(s_la[jp], 16)
                sync.dma_start(
                    out=abp_t[jp][:, 2 * ROW:4 * ROW],
                    in_=ab_d[r0 + P:r0 + 2 * P, :],
                ).then_inc(s_lb[jp], 16)
                if p == 0:
                    # tables after pair-0 data so compute starts sooner
                    sync.dma_start(
                        out=tabf_t[:], in_=tabf_d[:]).then_inc(s_tab, 16)
                elif p == 1:
                    sync.dma_start(
                        out=tabi_t[:], in_=tabi_d[:]).then_inc(s_tab2, 16)

        @block.gpsimd
        def _(gp: bass.BassEngine):
            for i in range(N_TILES):
                j = i % OBUF
                r0 = i * P
                gp.wait_ge(s_comp, 2 * i + 2)
                # SWDGE store with inline bf16 -> f32 upcast; legal here
                # because every DVE op in this kernel is TT/STT-class
                # (1-port), so descriptor generation is never locked out
                gp.dma_start(
                    out=out_d[:, r0:r0 + P, :].rearrange(
                        "two p c -> p two c"),
                    in_=out_t[j][:].rearrange("p (two c) -> p two c", two=2),
                ).then_inc(s_store[j], 16)

        @block.vector
        def _(vector: bass.BassEngine):
            # two-tile views: [p, tile, field]
            ivv = [iv8[c][:].rearrange("p (t f) -> p t f", f=4)
                   for c in range(2)]
            vvv = [v8[c][:].rearrange("p (t f) -> p t f", f=4)
                   for c in range(2)]

            vector.wait_ge(s_tab, 16)   # dec table (loaded first)
            for e in EVENTS:
                kind = e[0]
                if kind == "stt":
                    _, p, m = e
                    c = p % 2
                    jp = p % NPBUF
                    if m == 0:
                        if p >= 2:
                            # t8 parity reuse: pair p-2's cast consumed it
                            vector.wait_ge(s_dve, AFTER[("cast", p - 2)])
                        vector.wait_ge(s_la[jp], 16 * (p // NPBUF + 1))
                    elif m == 4:
                        vector.wait_ge(s_lb[jp], 16 * (p // NPBUF + 1))
                    off = 512 * m          # tile (m//4), segment (m%4)
                    vector.scalar_tensor_tensor(
                        out=dump[c][m][:],
                        in0=abp_t[jp][:, off:off + 512],
                        scalar=1.0,
                        in1=dec,
                        op0=mybir.AluOpType.mult,
                        op1=mybir.AluOpType.mult,
                        accum_out=t8[c][:, m:m + 1],
                    ).then_inc(s_dve, 1)
                elif kind == "cast":
                    _, q = e
                    c = q % 2
                    vector.wait_ge(s_dve, AFTER[("stt", q, 7)])
                    vector.tensor_copy(iv8[c][:], t8[c][:]).then_inc(s_dve, 1)
                elif kind == "add":
                    _, q = e
                    c = q % 2
                    vector.wait_ge(s_dve, AFTER[("cast", q)])
                    vector.tensor_tensor(
                        out=vvv[c][:, :, 0:2], in0=ivv[c][:, :, 0:2],
                        in1=ivv[c][:, :, 2:4],
                        op=mybir.AluOpType.add).then_inc(s_dve, 1)
                elif kind == "xor":
                    _, q = e
                    c = q % 2
                    vector.tensor_tensor(
                        out=vvv[c][:, :, 2:4], in0=ivv[c][:, :, 0:2],
                        in1=ivv[c][:, :, 2:4],
                        op=mybir.AluOpType.bitwise_xor).then_inc(s_dve, 1)
                elif kind == "isge":
                    _, q = e
                    c = q % 2
                    vector.wait_ge(s_dve, AFTER[("add", q)])
                    # carry lo->hi in place: s_hi += (s_lo >= 2^16)
                    vector.scalar_tensor_tensor(
                        out=vvv[c][:, :, 1:2], in0=vvv[c][:, :, 0:1],
                        scalar=65536,
                        in1=vvv[c][:, :, 1:2],
                        op0=mybir.AluOpType.is_ge,
                        op1=mybir.AluOpType.add).then_inc(s_dve, 1)
                elif kind == "shift":
                    _, q = e
                    c = q % 2
                    if q == 0:
                        vector.wait_ge(s_tab2, 16)  # shift/enc table ready
                    vector.wait_ge(s_dve, AFTER[("isge", q)])
                    vector.tensor_tensor(
                        out=sh16[c][:],
                        in0=v8[c][:, :, None].to_broadcast((P, 8, 2)),
                        in1=shifts,
                        op=mybir.AluOpType.logical_shift_right,
                    ).then_inc(s_dve, 1)
                elif kind == "and":
                    _, q = e
                    c = q % 2
                    vector.wait_ge(s_dve, AFTER[("shift", q)])
                    vector.tensor_scalar(
                        out=idx16[c][:], in0=sh16[c][:], scalar1=255,
                        scalar2=None,
                        op0=mybir.AluOpType.bitwise_and).then_inc(s_dve, 1)
                elif kind == "castb":
                    _, q = e
                    c = q % 2
                    if q >= 2:
                        # idx16bf parity reuse: pair q-2's encodes retired
                        vector.wait_ge(s_comp, 4 * (q - 1))
                    vector.wait_ge(s_dve, AFTER[("and", q)])
                    vector.tensor_copy(idx16bf[c][:], idx16[c][:]).then_inc(
                        s_dve, 1)
                else:  # iseq
                    _, t, h = e
                    c = (t // 2) % 2
                    jo = t % OBUF
                    off = 8 * (t % 2) + 4 * h
                    if h == 0:
                        vector.wait_ge(s_dve, AFTER[("castb", t // 2)])
                        if t >= OBUF:
                            vector.wait_ge(s_store[jo], 16 * (t // OBUF))
                    vector.tensor_tensor(
                        out=out_t[jo][:, ROW * h:ROW * (h + 1)].rearrange(
                            "p (e k) -> p e k", k=256),
                        in0=enc[:, 4 * h:4 * h + 4, :],
                        in1=idx16bf[c][:, off:off + 4, None].to_broadcast(
                            (P, 4, 256)),
                        op=mybir.AluOpType.is_equal,
                    ).then_inc(s_comp, 1)

    return nc


def _make_tables():
    dec = np.concatenate([np.arange(256), np.arange(256) * 256])
    enc = np.tile(np.arange(256), 8)
    tabf = np.tile(np.concatenate([dec, enc])[None, :], (P, 1)).astype(NP_BF16)
    shifts = np.array([0, 8] * 8, np.int64)
    tabi = np.tile(shifts.astype(np.int32)[None, :], (P, 1))
    return tabf, tabi


_NC_CACHE = {}


def _get_nc(variant: str = "main"):
    if variant not in _NC_CACHE:
        _NC_CACHE[variant] = _build_nc()
    return _NC_CACHE[variant]


def _pack_inputs(a: np.ndarray, b: np.ndarray) -> np.ndarray:
    """Interleave a|b rows and cast to bf16 (one-hot 0/1 is exact)."""
    ab = np.empty((B, 2 * ROW), NP_BF16)
    ab[:, 0:ROW] = a.reshape(B, ROW)
    ab[:, ROW:2 * ROW] = b.reshape(B, ROW)
    return ab


def _run(a: np.ndarray, b: np.ndarray, **spmd_kwargs):
    assert a.shape == (B, 4, 256) and b.shape == (B, 4, 256)
    ab = _pack_inputs(np.ascontiguousarray(a, dtype=np.float32),
                      np.ascontiguousarray(b, dtype=np.float32))
    tabf, tabi = _make_tables()
    in_maps = [
        {
            "ab": ab[i * B_LOC:(i + 1) * B_LOC],
            "tabf": tabf,
            "tabi": tabi,
        }
        for i in range(N_CORES)
    ]
    nc = _get_nc()
    kr = run_bass_kernel_spmd(nc, in_maps, list(range(N_CORES)), **spmd_kwargs)
    shards = [kr.results[i]["out"] for i in range(N_CORES)]
    out = np.concatenate(shards, axis=1).reshape(2, B, 4, 256)
    return out, kr


def kernel(a: np.ndarray, b: np.ndarray) -> np.ndarray:
    out, _ = _run(a, b)
    return out


# revision 20
# speedup vs baseline: 1.1509x; 1.1509x over previous
"""MoE-ALU (add with carry + xor over one-hot byte encodings) on 8 NeuronCores.

Semantics (validated against the jax reference bit-exactly): inputs a, b are
exact one-hot byte encodings [B, 4, 256] (little-endian bytes of 32-bit ints);
with SCALE=100 every softmax in the reference collapses to an exact one-hot, so

    out[0] = one_hot bytes of (a_int + b_int) mod 2^32
    out[1] = one_hot bytes of (a_int ^ b_int)

Device kernel (pure data parallel, batch sharded over 8 cores), raw Bass
(this toolchain's walrus encodes at most ONE sync wait per instruction, so
Tile-generated schedules don't compile; manual sems with standalone waits do).

The kernel is HBM-bound. The one-hot inputs are exactly representable in
bf16, so the host-side shard prep casts and interleaves a|b into one bf16
tensor: input traffic halves to 16 MB/core; the f32 output (32 MB) is
produced by casting bf16 one-hots inside the store DMA (SWDGE inline
upcast). Floor: ~48 MB/core of HBM wire time.

v7 structure -- two-tile pairs, software-pipelined DVE, three engines:

  load    two 512 KB DMAs per tile pair (bf16 a|b rows)          [SyncE]
  decode  per tile: 4x scalar_tensor_tensor with accum_out (multiply one
          512-col bf16 segment by the [0..255 | 0,256,...,65280] pattern,
          reduce in one op) -> a_lo a_hi b_lo b_hi              [VectorE]
  ints    per PAIR (strided APs): f32->i32 cast, add, xor, carry folded in
          place into the high halves ((s_lo>=2^16)+s_hi; the raw s_lo's
          bit 16 never survives >>0/>>8 then &255), shift/mask -> 16 byte
          indices, i32->f32 copy of the indices                 [VectorE]
  encode  ADD half: per byte, tensor_scalar is_equal of a bf16 iota[256]
          stream against the f32 per-partition index (AP scalar) -- the
          16-bit stream runs the DVE packed at 2x in single-port mode, so
          it never locks the DVE/GpSimd shared SBUF port        [VectorE]
          XOR half: one is_equal [128, 4, 256] broadcast-compare [GpSimd]
  store   one SWDGE DMA per tile with inline bf16 -> f32 upcast  [GpSimd]

  The emission order interleaves pair p's eight decode STTs between the
  dependent ops of pair p-1's int chain, so every RAW semaphore wait is
  already satisfied when the sequencer reaches it and the DVE never idles
  on sem latency (~130 ns per unsatisfied wait otherwise).

  ACT does nothing on purpose: measured on this part, a concurrent ACT
  stream slows every DVE op ~20%. GpSimd streaming is safe because every
  DVE op here is single-port (STT/TT 1x-2x_1p; the only 2-port-capable ops
  are sub-100ns copies), so the shared port pair is effectively free.

  DVE ops do NOT self-interlock (measured: removing sync gives stale reads),
  so every same-engine RAW step waits on the monotonically counted DVE
  semaphore; per-pair temporaries are parity-double-buffered. GpSimd's
  compute->store ordering is likewise guarded by its own s_gx counter.
"""
from contextlib import ExitStack

import ml_dtypes
import numpy as np

import concourse.bass as bass
from concourse import mybir
from concourse.bass_utils import run_bass_kernel_spmd

F32 = mybir.dt.float32
I32 = mybir.dt.int32
BF16 = mybir.dt.bfloat16
NP_BF16 = ml_dtypes.bfloat16

P = 128
N_CORES = 8
B = 32768
B_LOC = B // N_CORES          # 4096 rows per core
ROW = 4 * 256                 # 1024 elements per row per tensor
N_TILES = B_LOC // P          # 32
N_PAIRS = N_TILES // 2        # 16

NPBUF = 6                     # input buffer slots (one tile PAIR each)
OBUF = 10                     # output buffer slots

TABI_COLS = 16                # shift pattern x2 tiles
TABF_COLS = 512 + 2048        # decode pattern | encode iota x8 (bf16)


def _schedule():
    """Emission order for the vector engine (single source of truth for the
    s_dve counter, shared by the sync/vector closures)."""
    ev = []
    ev += [("stt", 0, m) for m in range(8)]
    tail = lambda q: [("iseq", 2 * q, ei) for ei in range(8)] + \
        [("iseq", 2 * q + 1, ei) for ei in range(8)]
    for p in range(1, N_PAIRS):
        q = p - 1
        ev += [
            ("stt", p, 0), ("cast", q),
            ("stt", p, 1), ("add", q),
            ("stt", p, 2), ("xor", q),
            ("stt", p, 3), ("isge", q),
            ("stt", p, 4), ("shift", q),
            ("stt", p, 5), ("and", q),
            ("stt", p, 6), ("castb", q),
            ("stt", p, 7),
        ] + tail(q)
    q = N_PAIRS - 1
    ev += [("cast", q), ("add", q), ("xor", q), ("isge", q), ("shift", q),
           ("and", q), ("castb", q)] + tail(q)

    after = {}
    n = 0
    for e in ev:
        if e[0] != "iseq":          # iseq increments s_comp, not s_dve
            n += 1
            after[e] = n
    return ev, after


EVENTS, AFTER = _schedule()
# input pair slot of pair p is free once its last decode STT retires
RELEASE_PAIR = {p: AFTER[("stt", p, 7)] for p in range(N_PAIRS)}


def _build_nc() -> bass.Bass:
    nc = bass.Bass(trn_type="TRN2")
    ab_d = nc.dram_tensor("ab", [B_LOC, 2 * ROW], BF16, kind="ExternalInput")
    tabf_d = nc.dram_tensor("tabf", [P, TABF_COLS], BF16, kind="ExternalInput")
    tabi_d = nc.dram_tensor("tabi", [P, TABI_COLS], I32, kind="ExternalInput")
    out_d = nc.dram_tensor("out", [2, B_LOC, ROW], F32, kind="ExternalOutput")

    with ExitStack() as ctx:
        sb = lambda name, shape, dt: ctx.enter_context(
            nc.sbuf_tensor(name, shape, dt))
        tabf_t = sb("tabf_t", [P, TABF_COLS], BF16)
        tabi_t = sb("tabi_t", [P, TABI_COLS], I32)
        abp_t = [sb(f"abp_t{k}", [P, 2 * 2 * ROW], BF16) for k in range(NPBUF)]
        out_t = [sb(f"out_t{k}", [P, 2 * ROW], BF16) for k in range(OBUF)]
        dump = [[sb(f"dump{c}_{k}", [P, 512], BF16) for k in range(8)]
                for c in range(2)]
        # parity-double-buffered per-pair temporaries
        t8 = [sb(f"t8_{c}", [P, 8], F32) for c in range(2)]
        iv8 = [sb(f"iv8_{c}", [P, 8], I32) for c in range(2)]
        v8 = [sb(f"v8_{c}", [P, 8], I32) for c in range(2)]
        sh16 = [sb(f"sh16_{c}", [P, 16], I32) for c in range(2)]
        idx16 = [sb(f"idx16_{c}", [P, 16], I32) for c in range(2)]
        idx16f = [sb(f"idx16f_{c}", [P, 16], F32) for c in range(2)]

        dec = tabf_t[:, 0:512]
        shifts = tabi_t[:].rearrange("p (a two) -> p a two", two=2)

        s_tab = ctx.enter_context(nc.semaphore("s_tab"))
        s_tab2 = ctx.enter_context(nc.semaphore("s_tab2"))
        s_tab3 = ctx.enter_context(nc.semaphore("s_tab3"))
        s_la = [ctx.enter_context(nc.semaphore(f"s_la{j}"))
                for j in range(NPBUF)]
        s_lb = [ctx.enter_context(nc.semaphore(f"s_lb{j}"))
                for j in range(NPBUF)]
        s_store = [ctx.enter_context(nc.semaphore(f"s_store{j}"))
                   for j in range(OBUF)]
        s_comp = ctx.enter_context(nc.semaphore("s_comp"))
        s_dve = ctx.enter_context(nc.semaphore("s_dve"))

        block = ctx.enter_context(nc.Block())

        @block.sync
        def _(sync: bass.BassEngine):
            for p in range(N_PAIRS):
                jp = p % NPBUF
                if p >= NPBUF:
                    sync.wait_ge(s_dve, RELEASE_PAIR[p - NPBUF])
                r0 = 2 * P * p
                sync.dma_start(
                    out=abp_t[jp][:, 0:2 * ROW],
                    in_=ab_d[r0:r0 + P, :],
                ).then_inc(s_la[jp], 16)
                if p == 0:
                    # decode table right after tile-0 data: compute starts
                    # as early as possible
                    sync.dma_start(
                        out=tabf_t[:, 0:512],
                        in_=tabf_d[:, 0:512]).then_inc(s_tab, 16)
                sync.dma_start(
                    out=abp_t[jp][:, 2 * ROW:4 * ROW],
                    in_=ab_d[r0 + P:r0 + 2 * P, :],
                ).then_inc(s_lb[jp], 16)
                if p == 0:
                    sync.dma_start(
                        out=tabf_t[:, 512:TABF_COLS],
                        in_=tabf_d[:, 512:TABF_COLS]).then_inc(s_tab3, 16)
                elif p == 1:
                    sync.dma_start(
                        out=tabi_t[:], in_=tabi_d[:]).then_inc(s_tab2, 16)

        @block.gpsimd
        def _(gp: bass.BassEngine):
            for i in range(N_TILES):
                j = i % OBUF
                r0 = i * P
                gp.wait_ge(s_comp, i + 1)     # both encode halves retired
                # SWDGE store with inline bf16 -> f32 upcast; safe alongside
                # the DVE because every DVE op here is single-port
                gp.dma_start(
                    out=out_d[:, r0:r0 + P, :].rearrange(
                        "two p c -> p two c"),
                    in_=out_t[j][:].rearrange("p (two c) -> p two c", two=2),
                ).then_inc(s_store[j], 16)

        @block.vector
        def _(vector: bass.BassEngine):
            # two-tile views: [p, tile, field]
            ivv = [iv8[c][:].rearrange("p (t f) -> p t f", f=4)
                   for c in range(2)]
            vvv = [v8[c][:].rearrange("p (t f) -> p t f", f=4)
                   for c in range(2)]

            vector.wait_ge(s_tab, 16)   # dec table (loaded first)
            for e in EVENTS:
                kind = e[0]
                if kind == "stt":
                    _, p, m = e
                    c = p % 2
                    jp = p % NPBUF
                    if m == 0:
                        if p >= 2:
                            # t8 parity reuse: pair p-2's cast consumed it
                            vector.wait_ge(s_dve, AFTER[("cast", p - 2)])
                        vector.wait_ge(s_la[jp], 16 * (p // NPBUF + 1))
                    elif m == 4:
                        vector.wait_ge(s_lb[jp], 16 * (p // NPBUF + 1))
                    off = 512 * m          # tile (m//4), segment (m%4)
                    vector.scalar_tensor_tensor(
                        out=dump[c][m][:],
                        in0=abp_t[jp][:, off:off + 512],
                        scalar=1.0,
                        in1=dec,
                        op0=mybir.AluOpType.mult,
                        op1=mybir.AluOpType.mult,
                        accum_out=t8[c][:, m:m + 1],
                    ).then_inc(s_dve, 1)
                elif kind == "cast":
                    _, q = e
                    c = q % 2
                    vector.wait_ge(s_dve, AFTER[("stt", q, 7)])
                    vector.tensor_copy(iv8[c][:], t8[c][:]).then_inc(s_dve, 1)
                elif kind == "add":
                    _, q = e
                    c = q % 2
                    vector.wait_ge(s_dve, AFTER[("cast", q)])
                    vector.tensor_tensor(
                        out=vvv[c][:, :, 0:2], in0=ivv[c][:, :, 0:2],
                        in1=ivv[c][:, :, 2:4],
                        op=mybir.AluOpType.add).then_inc(s_dve, 1)
                elif kind == "xor":
                    _, q = e
                    c = q % 2
                    vector.tensor_tensor(
                        out=vvv[c][:, :, 2:4], in0=ivv[c][:, :, 0:2],
                        in1=ivv[c][:, :, 2:4],
                        op=mybir.AluOpType.bitwise_xor).then_inc(s_dve, 1)
                elif kind == "isge":
                    _, q = e
                    c = q % 2
                    vector.wait_ge(s_dve, AFTER[("add", q)])
                    # carry lo->hi in place: s_hi += (s_lo >= 2^16)
                    vector.scalar_tensor_tensor(
                        out=vvv[c][:, :, 1:2], in0=vvv[c][:, :, 0:1],
                        scalar=65536,
                        in1=vvv[c][:, :, 1:2],
                        op0=mybir.AluOpType.is_ge,
                        op1=mybir.AluOpType.add).then_inc(s_dve, 1)
                elif kind == "shift":
                    _, q = e
                    c = q % 2
                    if q == 0:
                        vector.wait_ge(s_tab2, 16)  # shift table ready
                    vector.wait_ge(s_dve, AFTER[("isge", q)])
                    vector.tensor_tensor(
                        out=sh16[c][:],
                        in0=v8[c][:, :, None].to_broadcast((P, 8, 2)),
                        in1=shifts,
                        op=mybir.AluOpType.logical_shift_right,
                    ).then_inc(s_dve, 1)
                elif kind == "and":
                    _, q = e
                    c = q % 2
                    vector.wait_ge(s_dve, AFTER[("shift", q)])
                    vector.tensor_scalar(
                        out=idx16[c][:], in0=sh16[c][:], scalar1=255,
                        scalar2=None,
                        op0=mybir.AluOpType.bitwise_and).then_inc(s_dve, 1)
                elif kind == "castb":
                    _, q = e
                    c = q % 2
                    if q >= 2:
                        # idx16f parity reuse: pair q-2's encodes retired on
                        # both engines
                        vector.wait_ge(s_comp, 2 * (q - 2) + 2)
                    vector.wait_ge(s_dve, AFTER[("and", q)])
                    vector.tensor_copy(idx16f[c][:], idx16[c][:]).then_inc(
                        s_dve, 1)
                else:  # iseq: one output byte, bf16 iota vs f32 AP scalar
                    _, t, ei = e
                    c = (t // 2) % 2
                    jo = t % OBUF
                    if ei == 0:
                        vector.wait_ge(s_dve, AFTER[("castb", t // 2)])
                        if t == 0:
                            vector.wait_ge(s_tab3, 16)
                        if t >= OBUF:
                            vector.wait_ge(s_store[jo], 16 * (t // OBUF))
                    ins = vector.tensor_scalar(
                        out=out_t[jo][:, 256 * ei:256 * ei + 256],
                        in0=tabf_t[:, 512 + 256 * ei:768 + 256 * ei],
                        scalar1=idx16f[c][:, 8 * (t % 2) + ei:
                                          8 * (t % 2) + ei + 1],
                        scalar2=None,
                        op0=mybir.AluOpType.is_equal)
                    if ei == 7:
                        ins.then_inc(s_comp, 1)

    return nc


def _make_tables():
    dec = np.concatenate([np.arange(256), np.arange(256) * 256])
    enc = np.tile(np.arange(256), 8)
    tabf = np.tile(np.concatenate([dec, enc])[None, :], (P, 1)).astype(NP_BF16)
    shifts = np.array([0, 8] * 8, np.int64)
    tabi = np.tile(shifts.astype(np.int32)[None, :], (P, 1))
    return tabf, tabi


_NC_CACHE = {}


def _get_nc(variant: str = "main"):
    if variant not in _NC_CACHE:
        _NC_CACHE[variant] = _build_nc()
    return _NC_CACHE[variant]


def _pack_inputs(a: np.ndarray, b: np.ndarray) -> np.ndarray:
    """Interleave a|b rows and cast to bf16 (one-hot 0/1 is exact)."""
    ab = np.empty((B, 2 * ROW), NP_BF16)
    ab[:, 0:ROW] = a.reshape(B, ROW)
    ab[:, ROW:2 * ROW] = b.reshape(B, ROW)
    return ab


def _run(a: np.ndarray, b: np.ndarray, **spmd_kwargs):
    assert a.shape == (B, 4, 256) and b.shape == (B, 4, 256)
    ab = _pack_inputs(np.ascontiguousarray(a, dtype=np.float32),
                      np.ascontiguousarray(b, dtype=np.float32))
    tabf, tabi = _make_tables()
    in_maps = [
        {
            "ab": ab[i * B_LOC:(i + 1) * B_LOC],
            "tabf": tabf,
            "tabi": tabi,
        }
        for i in range(N_CORES)
    ]
    nc = _get_nc()
    kr = run_bass_kernel_spmd(nc, in_maps, list(range(N_CORES)), **spmd_kwargs)
    shards = [kr.results[i]["out"] for i in range(N_CORES)]
    out = np.concatenate(shards, axis=1).reshape(2, B, 4, 256)
    return out, kr


def kernel(a: np.ndarray, b: np.ndarray) -> np.ndarray:
    out, _ = _run(a, b)
    return out
